# revision 9
# baseline (speedup 1.0000x reference)
"""CGConv message-passing kernel for 8 Trainium2 NeuronCores.

Strategy (self-contained; shapes hardcoded for the nn_CGConv problem):
 - Sort edges by destination node (col); shard edges into 8 buckets by
   col-range so every core owns a disjoint slice of output nodes (no
   collective needed).
 - Host pre-gathers x[row]/x[col] as channel-major bf16 tables so the
   device streams them with plain contiguous DMA.
 - Per 512-edge tile the device computes gate/msg pre-activations with
   six N=512 matmuls (bias folded in via a ones-row on the edge_attr
   chunk), applies sigmoid x softplus via the natural_log_exp ACT table
   set only (exp(-g), exp(c), ln(t2+1); sigma = 1/(1+e^-g) on DVE), and
   segment-sums messages by destination via one-hot matmuls into a
   128-node window whose base is the tile's first (sorted) col.
 - Per-tile window sums go back to DRAM; the host merges windows,
   adds the residual, and assembles the full [25000, 128] output.
"""

import numpy as np
import ml_dtypes

BF16 = ml_dtypes.bfloat16

N_NODES = 25000
N_EDGES = 400000
C = 128
EC = 64
N_CORES = 8
SHARD = 3125          # nodes per core
SHARD_PAD = 3200      # padded (multiple of 128)
TILE = 512            # edges per super-tile


def _prep(x, edge_index, edge_attr, gate_w, gate_b, msg_w, msg_b):
    row = np.asarray(edge_index[0], dtype=np.int64)
    col = np.asarray(edge_index[1], dtype=np.int64)
    x = np.asarray(x, dtype=np.float32)
    edge_attr = np.asarray(edge_attr, dtype=np.float32)

    order = np.argsort(col, kind="stable")
    row_s, col_s = row[order], col[order]
    attr_s = edge_attr[order]

    shard_of = col_s // SHARD
    # bucket boundaries per core
    starts = np.searchsorted(shard_of, np.arange(N_CORES))
    ends = np.searchsorted(shard_of, np.arange(N_CORES), side="right")
    sizes = ends - starts
    e_pad = int(-(-sizes.max() // TILE) * TILE)
    n_sup = e_pad // TILE

    x_bf = x.astype(BF16)
    xT_bf = np.ascontiguousarray(x_bf.T)  # [128, N]

    # segment-window width: covers the max col-span of any 512-edge tile
    max_span = 0
    for i in range(N_CORES):
        c_l = col_s[starts[i]:ends[i]] - i * SHARD
        for t0 in range(0, len(c_l), TILE):
            seg = c_l[t0:t0 + TILE]
            max_span = max(max_span, int(seg[-1]) - int(seg[0]))
    Wseg = int(-(-(max_span + 3) // 32) * 32)  # pad-edge slack + round to 32
    Wseg = max(64, min(Wseg, 192))

    in_maps = []
    merge_info = []  # per core: (bases list, n_valid_edges)
    for i in range(N_CORES):
        sl = slice(starts[i], ends[i])
        r_i = row_s[sl]
        c_loc = (col_s[sl] - i * SHARD).astype(np.int64)
        a_i = attr_s[sl]
        n_i = sizes[i]
        pad = e_pad - n_i

        # pad col: must be >= SHARD (dropped at merge) and within W of its
        # tile's base. Real cols end <= 3124; pads go to a dedicated node id.
        if pad:
            last_real = int(c_loc[-1]) if n_i else 0
            pad_col = max(SHARD, last_real + 1)
            assert pad_col < SHARD_PAD
            r_i = np.concatenate([r_i, np.zeros(pad, np.int64)])
            c_loc = np.concatenate([c_loc, np.full(pad, pad_col, np.int64)])
            a_i = np.concatenate([a_i, np.zeros((pad, EC), np.float32)])

        # per-super-tile bases, clamped so base+W covers the tile's span
        bases = []
        for t in range(n_sup):
            seg = c_loc[t * TILE:(t + 1) * TILE]
            b = int(seg[0])
            assert int(seg[-1]) - b < Wseg, (
                f"core {i} tile {t}: col span {int(seg[-1]) - b} >= W={Wseg}")
            bases.append(b)
        bases_arr = np.repeat(np.asarray(bases, np.int64), TILE)

        xrowT = np.ascontiguousarray(x_bf[r_i].T)                  # [128, e_pad]
        # pad edges point past the real node range; clamp (their output is dropped)
        xcolT = np.ascontiguousarray(
            xT_bf[:, np.minimum(c_loc + i * SHARD, N_NODES - 1)])  # [128, e_pad]
        attrT = np.empty((EC + 1, e_pad), dtype=BF16)
        attrT[:EC] = a_i.T
        attrT[EC] = 1.0

        # one-hot segment matrices, host-built: B[e, w] = (col[e] - base == w)
        colsub = (c_loc - bases_arr).astype(np.int64)
        assert colsub.min() >= 0 and colsub.max() < Wseg
        bmat = np.zeros((e_pad, Wseg), dtype=BF16)
        bmat[np.arange(e_pad), colsub] = 1.0
        # [128 (edge-in-subtile), n_sub * W] so a tile's 4 B's are contiguous
        bmat = np.ascontiguousarray(
            bmat.reshape(e_pad // 128, 128, Wseg).transpose(1, 0, 2)
        ).reshape(128, -1)

        in_maps.append({
            "xrowT": xrowT,
            "xcolT": xcolT,
            "attrT": attrT,
            "bmat": bmat,
        })
        merge_info.append((bases, n_i))

    # shared (replicated) weight tables
    gw = np.asarray(gate_w, np.float32)
    mw = np.asarray(msg_w, np.float32)
    gb = np.asarray(gate_b, np.float32)
    mb = np.asarray(msg_b, np.float32)
    w1g = np.ascontiguousarray(gw[:, 0:128].T).astype(BF16)       # [128,128] lhsT
    w2g = np.ascontiguousarray(gw[:, 128:256].T).astype(BF16)
    no_bias = not gb.any() and not mb.any()
    if no_bias:
        # both attr projections ride one rhs via row-tiling: K rows 0-63
        # carry gate's attr chunk, rows 64-127 carry msg's.
        w3g = np.empty((C, C), dtype=BF16)
        w3g[:EC] = gw[:, 256:320].T
        w3g[EC:] = mw[:, 256:320].T
        w3m = w3g  # unused placeholder (same table)
    else:
        w3g = np.empty((EC + 1, C), dtype=BF16)
        w3g[:EC] = gw[:, 256:320].T
        w3g[EC] = gb
        w3m = np.empty((EC + 1, C), dtype=BF16)
        w3m[:EC] = mw[:, 256:320].T
        w3m[EC] = mb
    w1m = np.ascontiguousarray(mw[:, 0:128].T).astype(BF16)
    w2m = np.ascontiguousarray(mw[:, 128:256].T).astype(BF16)

    shared = {"w1g": w1g, "w2g": w2g, "w3g": w3g,
              "w1m": w1m, "w2m": w2m, "w3m": w3m}
    for m in in_maps:
        m.update(shared)

    meta = {"e_pad": e_pad, "n_sup": n_sup, "w_seg": Wseg, "no_bias": bool(no_bias)}
    return in_maps, meta, merge_info


def _build(meta):
    import concourse.bacc as bacc
    import concourse.mybir as mybir
    from concourse import tile

    e_pad, n_sup = meta["e_pad"], meta["n_sup"]
    W = meta["w_seg"]
    no_bias = meta["no_bias"]
    K3 = C if no_bias else EC + 1
    bf = mybir.dt.bfloat16
    f32 = mybir.dt.float32
    AF = mybir.ActivationFunctionType

    nc = bacc.Bacc(None, target_bir_lowering=False, debug=False)

    xrowT_d = nc.declare_dram_parameter("xrowT", [C, e_pad], bf, isOutput=False)
    xcolT_d = nc.declare_dram_parameter("xcolT", [C, e_pad], bf, isOutput=False)
    attrT_d = nc.declare_dram_parameter("attrT", [EC + 1, e_pad], bf, isOutput=False)
    bmat_d = nc.declare_dram_parameter("bmat", [128, (e_pad // 128) * W], bf, isOutput=False)
    w_d = {n: nc.declare_dram_parameter(n, [128 if n[1] in "12" else K3, C], bf, isOutput=False)
           for n in ("w1g", "w2g", "w3g", "w1m", "w2m", "w3m")}
    wsums_d = nc.declare_dram_parameter("wsums", [C, n_sup * W], f32, isOutput=True)

    with tile.TileContext(nc) as tc:
        with (
            tc.tile_pool(name="const", bufs=1) as cpool,
            tc.tile_pool(name="stream", bufs=3) as spool,
            tc.tile_pool(name="elem", bufs=3) as epool,
            tc.tile_pool(name="bmat", bufs=3) as bpool,
            tc.tile_pool(name="gps", bufs=3, space="PSUM") as gate_pool,
            tc.tile_pool(name="mps", bufs=3, space="PSUM") as msg_pool,
            tc.tile_pool(name="sps", bufs=2, space="PSUM") as s_pool,
        ):
            wt = {}
            for n, d in w_d.items():
                if no_bias and n == "w3m":
                    continue
                kdim = 128 if n[1] in "12" else K3
                wt[n] = cpool.tile([kdim, C], bf, tag=n, name=n)
                nc.sync.dma_start(wt[n][:], d[:])

            for t in range(n_sup):
                esl = slice(t * TILE, (t + 1) * TILE)
                xrow_t = spool.tile([C, TILE], bf, tag="xrow")
                nc.sync.dma_start(xrow_t[:], xrowT_d[:, esl])
                xcol_t = spool.tile([C, TILE], bf, tag="xcol")
                nc.sync.dma_start(xcol_t[:], xcolT_d[:, esl])
                if no_bias:
                    attr_t = spool.tile([C, TILE], bf, tag="attr")
                    nc.sync.dma_start(attr_t[0:EC, :], attrT_d[0:EC, esl])
                    nc.sync.dma_start(attr_t[EC:C, :], attrT_d[0:EC, esl])
                else:
                    attr_t = spool.tile([EC + 1, TILE], bf, tag="attr")
                    nc.sync.dma_start(attr_t[:], attrT_d[:, esl])

                gate_ps = gate_pool.tile([C, TILE], f32, tag="gate")
                msg_ps = msg_pool.tile([C, TILE], f32, tag="msg")
                nc.tensor.matmul(gate_ps[:], wt["w1g"][:], xrow_t[:], start=True, stop=False)
                nc.tensor.matmul(gate_ps[:], wt["w2g"][:], xcol_t[:], start=False, stop=False)
                nc.tensor.matmul(msg_ps[:], wt["w1m"][:], xrow_t[:], start=True, stop=False)
                nc.tensor.matmul(msg_ps[:], wt["w2m"][:], xcol_t[:], start=False, stop=False)
                if no_bias:
                    # both attr projections concurrently in disjoint row groups
                    nc.tensor.matmul(gate_ps[:], wt["w3g"][0:EC, :], attr_t[0:EC, :],
                                     start=False, stop=True, tile_position=(0, 0))
                    nc.tensor.matmul(msg_ps[:], wt["w3g"][EC:C, :], attr_t[EC:C, :],
                                     start=False, stop=True, tile_position=(64, 0))
                else:
                    nc.tensor.matmul(gate_ps[:], wt["w3g"][:], attr_t[:],
                                     start=False, stop=True)
                    nc.tensor.matmul(msg_ps[:], wt["w3m"][:], attr_t[:],
                                     start=False, stop=True)

                # t1 = exp(-g), t2 = exp(c), sp = ln(t2 + 1)
                t1 = epool.tile([C, TILE], f32, tag="t1")
                nc.scalar.activation(t1[:], gate_ps[:], AF.Exp, scale=-1.0)
                t2 = epool.tile([C, TILE], bf, tag="t2")
                nc.scalar.activation(t2[:], msg_ps[:], AF.Exp)
                sp = epool.tile([C, TILE], f32, tag="sp")
                nc.scalar.activation(sp[:], t2[:], AF.Ln, bias=1.0)

                # m = sp / (1 + t1)
                wd = epool.tile([C, TILE], f32, tag="wd")
                nc.vector.tensor_scalar_add(wd[:], t1[:], 1.0)
                rc = epool.tile([C, TILE], f32, tag="rc")
                nc.vector.reciprocal_approx_fast(rc[:], wd[:])
                m = epool.tile([C, TILE], bf, tag="m")
                nc.vector.tensor_mul(m[:], sp[:], rc[:])

                # transpose m -> [edges, ch] via the DMA xbar
                m_t = epool.tile([128, TILE], bf, tag="m_t")
                for j in range(4):
                    nc.sync.dma_start_transpose(m_t[:, j * 128:(j + 1) * 128],
                                                m[:, j * 128:(j + 1) * 128])

                # segment windows: host-built one-hot B, shared base per tile
                seg_ps = s_pool.tile([C, W], f32, tag="seg")
                b_t = bpool.tile([128, 4 * W], bf, tag="b")
                nc.sync.dma_start(b_t[:], bmat_d[:, t * 4 * W:(t + 1) * 4 * W])
                for j in range(4):
                    nc.tensor.matmul(seg_ps[:], m_t[:, j * 128:(j + 1) * 128],
                                     b_t[:, j * W:(j + 1) * W],
                                     start=(j == 0), stop=(j == 3))
                wsum = epool.tile([C, W], f32, tag="wsum")
                nc.vector.tensor_copy(wsum[:], seg_ps[:])
                nc.sync.dma_start(wsums_d[:, t * W:(t + 1) * W], wsum[:])

    # Force every activation onto natural_log_exp_and_others (exp AND ln):
    # the stock chooser greedily alternates exp_and_others/natural_log,
    # inserting ~2 table loads (~2.6us) per tile.
    import concourse.bacc as _bacc
    real_get = _bacc.get_activation_tables

    def pinned_tables(arch):
        tabs = real_get(arch)
        return {name: (funcs if name == "natural_log_exp_and_others" else set())
                for name, funcs in tabs.items()}

    _bacc.get_activation_tables = pinned_tables
    try:
        nc.compile()
    finally:
        _bacc.get_activation_tables = real_get
    return nc


def _postprocess(x, results, merge_info, meta):
    n_sup = meta["n_sup"]
    W = meta["w_seg"]
    out = np.empty((N_NODES, C), dtype=np.float32)
    for i in range(N_CORES):
        wsums = np.asarray(results[i]["wsums"], np.float32).reshape(C, n_sup, W)
        agg = np.zeros((C, SHARD_PAD + W), dtype=np.float32)
        bases, _ = merge_info[i]
        for t in range(n_sup):
            b = bases[t]
            agg[:, b:b + W] += wsums[:, t, :]
        out[i * SHARD:(i + 1) * SHARD] = agg[:, :SHARD].T
    out += np.asarray(x, np.float32)
    return out


_CACHE = {}


def kernel(**inputs):
    from concourse.bass_utils import run_bass_kernel_spmd

    in_maps, meta, merge_info = _prep(**inputs)
    key = (meta["e_pad"],)
    if key not in _CACHE:
        _CACHE[key] = _build(meta)
    nc = _CACHE[key]
    res = run_bass_kernel_spmd(nc, in_maps, core_ids=list(range(N_CORES)))
    return _postprocess(inputs["x"], res.results, merge_info, meta)


# revision 10
# speedup vs baseline: 3.1093x; 3.1093x over previous
"""CGConv message-passing kernel for 8 Trainium2 NeuronCores.

Strategy (self-contained; shapes hardcoded for the nn_CGConv problem):
 - Sort edges by destination node (col); shard edges into 8 buckets by
   col-range so every core owns a disjoint slice of output nodes (no
   collective needed).
 - Host pre-gathers x[row]/x[col] as channel-major bf16 tables so the
   device streams them with plain contiguous DMA.
 - Per 512-edge tile the device computes gate/msg pre-activations with
   six N=512 matmuls (bias folded in via a ones-row on the edge_attr
   chunk), applies sigmoid x softplus via the natural_log_exp ACT table
   set only (exp(-g), exp(c), ln(t2+1); sigma = 1/(1+e^-g) on DVE), and
   segment-sums messages by destination via one-hot matmuls into a
   128-node window whose base is the tile's first (sorted) col.
 - Per-tile window sums go back to DRAM; the host merges windows,
   adds the residual, and assembles the full [25000, 128] output.
"""

import numpy as np
import ml_dtypes

BF16 = ml_dtypes.bfloat16

N_NODES = 25000
N_EDGES = 400000
C = 128
EC = 64
N_CORES = 8
SHARD = 3125          # nodes per core
SHARD_PAD = 3200      # padded (multiple of 128)
TILE = 512            # edges per super-tile


def _prep(x, edge_index, edge_attr, gate_w, gate_b, msg_w, msg_b):
    row = np.asarray(edge_index[0], dtype=np.int64)
    col = np.asarray(edge_index[1], dtype=np.int64)
    x = np.asarray(x, dtype=np.float32)
    edge_attr = np.asarray(edge_attr, dtype=np.float32)

    order = np.argsort(col, kind="stable")
    row_s, col_s = row[order], col[order]
    attr_s = edge_attr[order]

    shard_of = col_s // SHARD
    # bucket boundaries per core
    starts = np.searchsorted(shard_of, np.arange(N_CORES))
    ends = np.searchsorted(shard_of, np.arange(N_CORES), side="right")
    sizes = ends - starts
    e_pad = int(-(-sizes.max() // TILE) * TILE)
    n_sup = e_pad // TILE

    x_bf = x.astype(BF16)
    xT_bf = np.ascontiguousarray(x_bf.T)  # [128, N]

    # segment-window width: covers the max col-span of any 512-edge tile
    max_span = 0
    for i in range(N_CORES):
        c_l = col_s[starts[i]:ends[i]] - i * SHARD
        for t0 in range(0, len(c_l), TILE):
            seg = c_l[t0:t0 + TILE]
            max_span = max(max_span, int(seg[-1]) - int(seg[0]))
    Wseg = int(-(-(max_span + 3) // 32) * 32)  # pad-edge slack + round to 32
    Wseg = max(64, min(Wseg, 192))

    in_maps = []
    merge_info = []  # per core: (bases list, n_valid_edges)
    for i in range(N_CORES):
        sl = slice(starts[i], ends[i])
        r_i = row_s[sl]
        c_loc = (col_s[sl] - i * SHARD).astype(np.int64)
        a_i = attr_s[sl]
        n_i = sizes[i]
        pad = e_pad - n_i

        # pad col: must be >= SHARD (dropped at merge) and within W of its
        # tile's base. Real cols end <= 3124; pads go to a dedicated node id.
        if pad:
            last_real = int(c_loc[-1]) if n_i else 0
            pad_col = max(SHARD, last_real + 1)
            assert pad_col < SHARD_PAD
            r_i = np.concatenate([r_i, np.zeros(pad, np.int64)])
            c_loc = np.concatenate([c_loc, np.full(pad, pad_col, np.int64)])
            a_i = np.concatenate([a_i, np.zeros((pad, EC), np.float32)])

        # per-super-tile bases, clamped so base+W covers the tile's span
        bases = []
        for t in range(n_sup):
            seg = c_loc[t * TILE:(t + 1) * TILE]
            b = int(seg[0])
            assert int(seg[-1]) - b < Wseg, (
                f"core {i} tile {t}: col span {int(seg[-1]) - b} >= W={Wseg}")
            bases.append(b)
        bases_arr = np.repeat(np.asarray(bases, np.int64), TILE)

        xrowT = np.ascontiguousarray(x_bf[r_i].T)                  # [128, e_pad]
        # pad edges point past the real node range; clamp (their output is dropped)
        xcolT = np.ascontiguousarray(
            xT_bf[:, np.minimum(c_loc + i * SHARD, N_NODES - 1)])  # [128, e_pad]
        attrT = np.empty((EC + 1, e_pad), dtype=BF16)
        attrT[:EC] = a_i.T
        attrT[EC] = 1.0

        # one-hot segment matrices, host-built: B[e, w] = (col[e] - base == w)
        colsub = (c_loc - bases_arr).astype(np.int64)
        assert colsub.min() >= 0 and colsub.max() < Wseg
        bmat = np.zeros((e_pad, Wseg), dtype=BF16)
        bmat[np.arange(e_pad), colsub] = 1.0
        # [128 (edge-in-subtile), n_sub * W] so a tile's 4 B's are contiguous
        bmat = np.ascontiguousarray(
            bmat.reshape(e_pad // 128, 128, Wseg).transpose(1, 0, 2)
        ).reshape(128, -1)

        in_maps.append({
            "xrowT": xrowT,
            "xcolT": xcolT,
            "attrT": attrT,
            "bmat": bmat,
        })
        merge_info.append((bases, n_i))

    # shared (replicated) weight tables
    gw = np.asarray(gate_w, np.float32)
    mw = np.asarray(msg_w, np.float32)
    gb = np.asarray(gate_b, np.float32)
    mb = np.asarray(msg_b, np.float32)
    w1g = np.ascontiguousarray(gw[:, 0:128].T).astype(BF16)       # [128,128] lhsT
    w2g = np.ascontiguousarray(gw[:, 128:256].T).astype(BF16)
    no_bias = not gb.any() and not mb.any()
    if no_bias:
        # both attr projections ride one rhs via row-tiling: K rows 0-63
        # carry gate's attr chunk, rows 64-127 carry msg's.
        w3g = np.empty((C, C), dtype=BF16)
        w3g[:EC] = gw[:, 256:320].T
        w3g[EC:] = mw[:, 256:320].T
        w3m = w3g  # unused placeholder (same table)
    else:
        w3g = np.empty((EC + 1, C), dtype=BF16)
        w3g[:EC] = gw[:, 256:320].T
        w3g[EC] = gb
        w3m = np.empty((EC + 1, C), dtype=BF16)
        w3m[:EC] = mw[:, 256:320].T
        w3m[EC] = mb
    w1m = np.ascontiguousarray(mw[:, 0:128].T).astype(BF16)
    w2m = np.ascontiguousarray(mw[:, 128:256].T).astype(BF16)

    ident = np.eye(128, dtype=BF16)
    shared = {"w1g": w1g, "w2g": w2g, "w3g": w3g,
              "w1m": w1m, "w2m": w2m, "w3m": w3m, "ident": ident}
    for m in in_maps:
        m.update(shared)

    meta = {"e_pad": e_pad, "n_sup": n_sup, "w_seg": Wseg, "no_bias": bool(no_bias)}
    return in_maps, meta, merge_info


def _build(meta):
    import concourse.bacc as bacc
    import concourse.mybir as mybir
    from concourse import tile

    e_pad, n_sup = meta["e_pad"], meta["n_sup"]
    W = meta["w_seg"]
    no_bias = meta["no_bias"]
    K3 = C if no_bias else EC + 1
    bf = mybir.dt.bfloat16
    f32 = mybir.dt.float32
    AF = mybir.ActivationFunctionType

    nc = bacc.Bacc(None, target_bir_lowering=False, debug=False)

    xrowT_d = nc.declare_dram_parameter("xrowT", [C, e_pad], bf, isOutput=False)
    xcolT_d = nc.declare_dram_parameter("xcolT", [C, e_pad], bf, isOutput=False)
    attrT_d = nc.declare_dram_parameter("attrT", [EC + 1, e_pad], bf, isOutput=False)
    bmat_d = nc.declare_dram_parameter("bmat", [128, (e_pad // 128) * W], bf, isOutput=False)
    ident_d = nc.declare_dram_parameter("ident", [128, 128], bf, isOutput=False)
    w_d = {n: nc.declare_dram_parameter(n, [128 if n[1] in "12" else K3, C], bf, isOutput=False)
           for n in ("w1g", "w2g", "w3g", "w1m", "w2m", "w3m")}
    wsums_d = nc.declare_dram_parameter("wsums", [C, n_sup * W], f32, isOutput=True)

    with tile.TileContext(nc) as tc:
        with (
            tc.tile_pool(name="const", bufs=1) as cpool,
            tc.tile_pool(name="stream", bufs=3) as spool,
            tc.tile_pool(name="elem", bufs=3) as epool,
            tc.tile_pool(name="bmat", bufs=3) as bpool,
            tc.tile_pool(name="gps", bufs=2, space="PSUM") as gate_pool,
            tc.tile_pool(name="mps", bufs=2, space="PSUM") as msg_pool,
            tc.tile_pool(name="tps", bufs=2, space="PSUM") as t_pool,
            tc.tile_pool(name="sps", bufs=2, space="PSUM") as s_pool,
        ):
            wt = {}
            ident = cpool.tile([128, 128], bf, tag="ident")
            nc.sync.dma_start(ident[:], ident_d[:])
            for n, d in w_d.items():
                if no_bias and n == "w3m":
                    continue
                kdim = 128 if n[1] in "12" else K3
                wt[n] = cpool.tile([kdim, C], bf, tag=n, name=n)
                nc.sync.dma_start(wt[n][:], d[:])

            for t in range(n_sup):
                esl = slice(t * TILE, (t + 1) * TILE)
                xrow_t = spool.tile([C, TILE], bf, tag="xrow")
                nc.sync.dma_start(xrow_t[:], xrowT_d[:, esl])
                xcol_t = spool.tile([C, TILE], bf, tag="xcol")
                nc.sync.dma_start(xcol_t[:], xcolT_d[:, esl])
                if no_bias:
                    attr_t = spool.tile([C, TILE], bf, tag="attr")
                    nc.sync.dma_start(attr_t[0:EC, :], attrT_d[0:EC, esl])
                    nc.sync.dma_start(attr_t[EC:C, :], attrT_d[0:EC, esl])
                else:
                    attr_t = spool.tile([EC + 1, TILE], bf, tag="attr")
                    nc.sync.dma_start(attr_t[:], attrT_d[:, esl])

                gate_ps = gate_pool.tile([C, TILE], f32, tag="gate")
                msg_ps = msg_pool.tile([C, TILE], f32, tag="msg")
                nc.tensor.matmul(gate_ps[:], wt["w1g"][:], xrow_t[:], start=True, stop=False)
                nc.tensor.matmul(gate_ps[:], wt["w2g"][:], xcol_t[:], start=False, stop=False)
                nc.tensor.matmul(msg_ps[:], wt["w1m"][:], xrow_t[:], start=True, stop=False)
                nc.tensor.matmul(msg_ps[:], wt["w2m"][:], xcol_t[:], start=False, stop=False)
                if no_bias:
                    # both attr projections concurrently in disjoint row groups
                    nc.tensor.matmul(gate_ps[:], wt["w3g"][0:EC, :], attr_t[0:EC, :],
                                     start=False, stop=True, tile_position=(0, 0))
                    nc.tensor.matmul(msg_ps[:], wt["w3g"][EC:C, :], attr_t[EC:C, :],
                                     start=False, stop=True, tile_position=(64, 0))
                else:
                    nc.tensor.matmul(gate_ps[:], wt["w3g"][:], attr_t[:],
                                     start=False, stop=True)
                    nc.tensor.matmul(msg_ps[:], wt["w3m"][:], attr_t[:],
                                     start=False, stop=True)

                # t1 = exp(-g), t2 = exp(c), sp = ln(t2 + 1)
                t1 = epool.tile([C, TILE], f32, tag="t1")
                nc.scalar.activation(t1[:], gate_ps[:], AF.Exp, scale=-1.0)
                t2 = epool.tile([C, TILE], bf, tag="t2")
                nc.scalar.activation(t2[:], msg_ps[:], AF.Exp)
                sp = epool.tile([C, TILE], f32, tag="sp")
                nc.scalar.activation(sp[:], t2[:], AF.Ln, bias=1.0)

                # m = sp / (1 + t1)
                wd = epool.tile([C, TILE], f32, tag="wd")
                nc.vector.tensor_scalar_add(wd[:], t1[:], 1.0)
                rc = epool.tile([C, TILE], f32, tag="rc")
                nc.vector.reciprocal_approx_fast(rc[:], wd[:])
                m = epool.tile([C, TILE], bf, tag="m")
                nc.vector.tensor_mul(m[:], sp[:], rc[:])

                # transpose m -> [edges, ch] on the PE
                mt_ps = t_pool.tile([128, TILE], bf, tag="mt")
                for j in range(4):
                    nc.tensor.transpose(mt_ps[:, j * 128:(j + 1) * 128],
                                        m[:, j * 128:(j + 1) * 128], ident[:])
                m_t = epool.tile([128, TILE], bf, tag="m_t")
                nc.vector.tensor_copy(m_t[:], mt_ps[:])

                # segment windows: host-built one-hot B, shared base per tile
                seg_ps = s_pool.tile([C, W], f32, tag="seg")
                b_t = bpool.tile([128, 4 * W], bf, tag="b")
                nc.sync.dma_start(b_t[:], bmat_d[:, t * 4 * W:(t + 1) * 4 * W])
                for j in range(4):
                    nc.tensor.matmul(seg_ps[:], m_t[:, j * 128:(j + 1) * 128],
                                     b_t[:, j * W:(j + 1) * W],
                                     start=(j == 0), stop=(j == 3))
                wsum = epool.tile([C, W], f32, tag="wsum")
                nc.vector.tensor_copy(wsum[:], seg_ps[:])
                nc.sync.dma_start(wsums_d[:, t * W:(t + 1) * W], wsum[:])

    # Force every activation onto natural_log_exp_and_others (exp AND ln):
    # the stock chooser greedily alternates exp_and_others/natural_log,
    # inserting ~2 table loads (~2.6us) per tile.
    import concourse.bacc as _bacc
    real_get = _bacc.get_activation_tables

    def pinned_tables(arch):
        tabs = real_get(arch)
        return {name: (funcs if name == "natural_log_exp_and_others" else set())
                for name, funcs in tabs.items()}

    _bacc.get_activation_tables = pinned_tables
    try:
        nc.compile()
    finally:
        _bacc.get_activation_tables = real_get
    return nc


def _postprocess(x, results, merge_info, meta):
    n_sup = meta["n_sup"]
    W = meta["w_seg"]
    out = np.empty((N_NODES, C), dtype=np.float32)
    for i in range(N_CORES):
        wsums = np.asarray(results[i]["wsums"], np.float32).reshape(C, n_sup, W)
        agg = np.zeros((C, SHARD_PAD + W), dtype=np.float32)
        bases, _ = merge_info[i]
        for t in range(n_sup):
            b = bases[t]
            agg[:, b:b + W] += wsums[:, t, :]
        out[i * SHARD:(i + 1) * SHARD] = agg[:, :SHARD].T
    out += np.asarray(x, np.float32)
    return out


_CACHE = {}


def kernel(**inputs):
    from concourse.bass_utils import run_bass_kernel_spmd

    in_maps, meta, merge_info = _prep(**inputs)
    key = (meta["e_pad"],)
    if key not in _CACHE:
        _CACHE[key] = _build(meta)
    nc = _CACHE[key]
    res = run_bass_kernel_spmd(nc, in_maps, core_ids=list(range(N_CORES)))
    return _postprocess(inputs["x"], res.results, merge_info, meta)


# revision 11
# speedup vs baseline: 3.1835x; 1.0239x over previous
"""CGConv message-passing kernel for 8 Trainium2 NeuronCores.

Strategy (self-contained; shapes hardcoded for the nn_CGConv problem):
 - Sort edges by destination node (col); shard edges into 8 buckets by
   col-range so every core owns a disjoint slice of output nodes (no
   collective needed).
 - Host pre-gathers x[row]/x[col] as channel-major bf16 tables so the
   device streams them with plain contiguous DMA.
 - Per 512-edge tile the device computes gate/msg pre-activations with
   six N=512 matmuls (bias folded in via a ones-row on the edge_attr
   chunk), applies sigmoid x softplus via the natural_log_exp ACT table
   set only (exp(-g), exp(c), ln(t2+1); sigma = 1/(1+e^-g) on DVE), and
   segment-sums messages by destination via one-hot matmuls into a
   128-node window whose base is the tile's first (sorted) col.
 - Per-tile window sums go back to DRAM; the host merges windows,
   adds the residual, and assembles the full [25000, 128] output.
"""

import numpy as np
import ml_dtypes

BF16 = ml_dtypes.bfloat16

N_NODES = 25000
N_EDGES = 400000
C = 128
EC = 64
N_CORES = 8
SHARD = 3125          # nodes per core
SHARD_PAD = 3200      # padded (multiple of 128)
TILE = 512            # edges per super-tile


def _prep(x, edge_index, edge_attr, gate_w, gate_b, msg_w, msg_b):
    row = np.asarray(edge_index[0], dtype=np.int64)
    col = np.asarray(edge_index[1], dtype=np.int64)
    x = np.asarray(x, dtype=np.float32)
    edge_attr = np.asarray(edge_attr, dtype=np.float32)

    order = np.argsort(col, kind="stable")
    row_s, col_s = row[order], col[order]
    attr_s = edge_attr[order]

    shard_of = col_s // SHARD
    # bucket boundaries per core
    starts = np.searchsorted(shard_of, np.arange(N_CORES))
    ends = np.searchsorted(shard_of, np.arange(N_CORES), side="right")
    sizes = ends - starts
    e_pad = int(-(-sizes.max() // TILE) * TILE)
    n_sup = e_pad // TILE

    x_bf = x.astype(BF16)
    xT_bf = np.ascontiguousarray(x_bf.T)  # [128, N]

    # segment-window width: covers the max col-span of any 512-edge tile
    max_span = 0
    for i in range(N_CORES):
        c_l = col_s[starts[i]:ends[i]] - i * SHARD
        for t0 in range(0, len(c_l), TILE):
            seg = c_l[t0:t0 + TILE]
            max_span = max(max_span, int(seg[-1]) - int(seg[0]))
    Wseg = int(-(-(max_span + 3) // 32) * 32)  # pad-edge slack + round to 32
    Wseg = max(64, min(Wseg, 192))

    in_maps = []
    merge_info = []  # per core: (bases list, n_valid_edges)
    for i in range(N_CORES):
        sl = slice(starts[i], ends[i])
        r_i = row_s[sl]
        c_loc = (col_s[sl] - i * SHARD).astype(np.int64)
        a_i = attr_s[sl]
        n_i = sizes[i]
        pad = e_pad - n_i

        # pad col: must be >= SHARD (dropped at merge) and within W of its
        # tile's base. Real cols end <= 3124; pads go to a dedicated node id.
        if pad:
            last_real = int(c_loc[-1]) if n_i else 0
            pad_col = max(SHARD, last_real + 1)
            assert pad_col < SHARD_PAD
            r_i = np.concatenate([r_i, np.zeros(pad, np.int64)])
            c_loc = np.concatenate([c_loc, np.full(pad, pad_col, np.int64)])
            a_i = np.concatenate([a_i, np.zeros((pad, EC), np.float32)])

        # per-super-tile bases, clamped so base+W covers the tile's span
        bases = []
        for t in range(n_sup):
            seg = c_loc[t * TILE:(t + 1) * TILE]
            b = int(seg[0])
            assert int(seg[-1]) - b < Wseg, (
                f"core {i} tile {t}: col span {int(seg[-1]) - b} >= W={Wseg}")
            bases.append(b)
        bases_arr = np.repeat(np.asarray(bases, np.int64), TILE)

        xrowT = np.ascontiguousarray(x_bf[r_i].T)                  # [128, e_pad]
        # pad edges point past the real node range; clamp (their output is dropped)
        xcolT = np.ascontiguousarray(
            xT_bf[:, np.minimum(c_loc + i * SHARD, N_NODES - 1)])  # [128, e_pad]
        attrT = np.empty((EC + 1, e_pad), dtype=BF16)
        attrT[:EC] = a_i.T
        attrT[EC] = 1.0

        # one-hot segment matrices, host-built: B[e, w] = (col[e] - base == w)
        colsub = (c_loc - bases_arr).astype(np.int64)
        assert colsub.min() >= 0 and colsub.max() < Wseg
        bmat = np.zeros((e_pad, Wseg), dtype=BF16)
        bmat[np.arange(e_pad), colsub] = 1.0
        # [128 (edge-in-subtile), n_sub * W] so a tile's 4 B's are contiguous
        bmat = np.ascontiguousarray(
            bmat.reshape(e_pad // 128, 128, Wseg).transpose(1, 0, 2)
        ).reshape(128, -1)

        in_maps.append({
            "xrowT": xrowT,
            "xcolT": xcolT,
            "attrT": attrT,
            "bmat": bmat,
        })
        merge_info.append((bases, n_i))

    # shared (replicated) weight tables
    gw = np.asarray(gate_w, np.float32)
    mw = np.asarray(msg_w, np.float32)
    gb = np.asarray(gate_b, np.float32)
    mb = np.asarray(msg_b, np.float32)
    w1g = np.ascontiguousarray(gw[:, 0:128].T).astype(BF16)       # [128,128] lhsT
    w2g = np.ascontiguousarray(gw[:, 128:256].T).astype(BF16)
    no_bias = False
    w3g = np.empty((EC + 1, C), dtype=BF16)
    w3g[:EC] = gw[:, 256:320].T
    w3g[EC] = gb
    w3m = np.empty((EC + 1, C), dtype=BF16)
    w3m[:EC] = mw[:, 256:320].T
    w3m[EC] = mb
    w1m = np.ascontiguousarray(mw[:, 0:128].T).astype(BF16)
    w2m = np.ascontiguousarray(mw[:, 128:256].T).astype(BF16)

    ident = np.eye(128, dtype=BF16)
    shared = {"w1g": w1g, "w2g": w2g, "w3g": w3g,
              "w1m": w1m, "w2m": w2m, "w3m": w3m, "ident": ident}
    for m in in_maps:
        m.update(shared)

    meta = {"e_pad": e_pad, "n_sup": n_sup, "w_seg": Wseg, "no_bias": bool(no_bias)}
    return in_maps, meta, merge_info


def _build(meta):
    import concourse.bacc as bacc
    import concourse.mybir as mybir
    from concourse import tile

    e_pad, n_sup = meta["e_pad"], meta["n_sup"]
    W = meta["w_seg"]
    no_bias = meta["no_bias"]
    K3 = C if no_bias else EC + 1
    bf = mybir.dt.bfloat16
    f32 = mybir.dt.float32
    AF = mybir.ActivationFunctionType

    nc = bacc.Bacc(None, target_bir_lowering=False, debug=False)

    xrowT_d = nc.declare_dram_parameter("xrowT", [C, e_pad], bf, isOutput=False)
    xcolT_d = nc.declare_dram_parameter("xcolT", [C, e_pad], bf, isOutput=False)
    attrT_d = nc.declare_dram_parameter("attrT", [EC + 1, e_pad], bf, isOutput=False)
    bmat_d = nc.declare_dram_parameter("bmat", [128, (e_pad // 128) * W], bf, isOutput=False)
    ident_d = nc.declare_dram_parameter("ident", [128, 128], bf, isOutput=False)
    w_d = {n: nc.declare_dram_parameter(n, [128 if n[1] in "12" else K3, C], bf, isOutput=False)
           for n in ("w1g", "w2g", "w3g", "w1m", "w2m", "w3m")}
    wsums_d = nc.declare_dram_parameter("wsums", [C, n_sup * W], f32, isOutput=True)

    with tile.TileContext(nc) as tc:
        with (
            tc.tile_pool(name="const", bufs=1) as cpool,
            tc.tile_pool(name="stream", bufs=3) as spool,
            tc.tile_pool(name="elem", bufs=3) as epool,
            tc.tile_pool(name="bmat", bufs=3) as bpool,
            tc.tile_pool(name="gps", bufs=2, space="PSUM") as gate_pool,
            tc.tile_pool(name="mps", bufs=2, space="PSUM") as msg_pool,
            tc.tile_pool(name="tps", bufs=2, space="PSUM") as t_pool,
            tc.tile_pool(name="sps", bufs=2, space="PSUM") as s_pool,
        ):
            wt = {}
            ident = cpool.tile([128, 128], bf, tag="ident")
            nc.sync.dma_start(ident[:], ident_d[:])
            for n, d in w_d.items():
                if no_bias and n == "w3m":
                    continue
                kdim = 128 if n[1] in "12" else K3
                wt[n] = cpool.tile([kdim, C], bf, tag=n, name=n)
                nc.sync.dma_start(wt[n][:], d[:])

            for t in range(n_sup):
                esl = slice(t * TILE, (t + 1) * TILE)
                xrow_t = spool.tile([C, TILE], bf, tag="xrow")
                nc.sync.dma_start(xrow_t[:], xrowT_d[:, esl])
                xcol_t = spool.tile([C, TILE], bf, tag="xcol")
                nc.sync.dma_start(xcol_t[:], xcolT_d[:, esl])
                attr_t = spool.tile([EC + 1, TILE], bf, tag="attr")
                nc.sync.dma_start(attr_t[:], attrT_d[:, esl])

                gate_ps = gate_pool.tile([C, TILE], f32, tag="gate")
                msg_ps = msg_pool.tile([C, TILE], f32, tag="msg")
                nc.tensor.matmul(gate_ps[:], wt["w1g"][:], xrow_t[:], start=True, stop=False)
                nc.tensor.matmul(gate_ps[:], wt["w2g"][:], xcol_t[:], start=False, stop=False)
                nc.tensor.matmul(msg_ps[:], wt["w1m"][:], xrow_t[:], start=True, stop=False)
                nc.tensor.matmul(msg_ps[:], wt["w2m"][:], xcol_t[:], start=False, stop=False)
                nc.tensor.matmul(gate_ps[:], wt["w3g"][:], attr_t[:],
                                 start=False, stop=True)
                nc.tensor.matmul(msg_ps[:], wt["w3m"][:], attr_t[:],
                                 start=False, stop=True)

                # t1 = exp(-g), t2 = exp(c), sp = ln(t2 + 1)
                t1 = epool.tile([C, TILE], f32, tag="t1")
                nc.scalar.activation(t1[:], gate_ps[:], AF.Exp, scale=-1.0)
                t2 = epool.tile([C, TILE], bf, tag="t2")
                nc.scalar.activation(t2[:], msg_ps[:], AF.Exp)
                sp = epool.tile([C, TILE], f32, tag="sp")
                nc.scalar.activation(sp[:], t2[:], AF.Ln, bias=1.0)

                # m = sp / (1 + t1)
                wd = epool.tile([C, TILE], f32, tag="wd")
                nc.vector.tensor_scalar_add(wd[:], t1[:], 1.0)
                rc = epool.tile([C, TILE], f32, tag="rc")
                nc.vector.reciprocal_approx_fast(rc[:], wd[:])
                m = epool.tile([C, TILE], bf, tag="m")
                nc.vector.tensor_mul(m[:], sp[:], rc[:])

                # transpose m -> [edges, ch] on the PE
                mt_ps = t_pool.tile([128, TILE], bf, tag="mt")
                for j in range(4):
                    nc.tensor.transpose(mt_ps[:, j * 128:(j + 1) * 128],
                                        m[:, j * 128:(j + 1) * 128], ident[:])
                m_t = epool.tile([128, TILE], bf, tag="m_t")
                nc.vector.tensor_copy(m_t[:], mt_ps[:])

                # segment windows: host-built one-hot B, shared base per tile
                seg_ps = s_pool.tile([C, W], f32, tag="seg")
                b_t = bpool.tile([128, 4 * W], bf, tag="b")
                nc.sync.dma_start(b_t[:], bmat_d[:, t * 4 * W:(t + 1) * 4 * W])
                for j in range(4):
                    nc.tensor.matmul(seg_ps[:], m_t[:, j * 128:(j + 1) * 128],
                                     b_t[:, j * W:(j + 1) * W],
                                     start=(j == 0), stop=(j == 3))
                wsum = epool.tile([C, W], f32, tag="wsum")
                nc.vector.tensor_copy(wsum[:], seg_ps[:])
                nc.sync.dma_start(wsums_d[:, t * W:(t + 1) * W], wsum[:])

    # Force every activation onto natural_log_exp_and_others (exp AND ln):
    # the stock chooser greedily alternates exp_and_others/natural_log,
    # inserting ~2 table loads (~2.6us) per tile.
    import concourse.bacc as _bacc
    real_get = _bacc.get_activation_tables

    def pinned_tables(arch):
        tabs = real_get(arch)
        return {name: (funcs if name == "natural_log_exp_and_others" else set())
                for name, funcs in tabs.items()}

    _bacc.get_activation_tables = pinned_tables
    try:
        nc.compile()
    finally:
        _bacc.get_activation_tables = real_get
    return nc


def _postprocess(x, results, merge_info, meta):
    n_sup = meta["n_sup"]
    W = meta["w_seg"]
    out = np.empty((N_NODES, C), dtype=np.float32)
    for i in range(N_CORES):
        wsums = np.asarray(results[i]["wsums"], np.float32).reshape(C, n_sup, W)
        agg = np.zeros((C, SHARD_PAD + W), dtype=np.float32)
        bases, _ = merge_info[i]
        for t in range(n_sup):
            b = bases[t]
            agg[:, b:b + W] += wsums[:, t, :]
        out[i * SHARD:(i + 1) * SHARD] = agg[:, :SHARD].T
    out += np.asarray(x, np.float32)
    return out


_CACHE = {}


def kernel(**inputs):
    from concourse.bass_utils import run_bass_kernel_spmd

    in_maps, meta, merge_info = _prep(**inputs)
    key = (meta["e_pad"],)
    if key not in _CACHE:
        _CACHE[key] = _build(meta)
    nc = _CACHE[key]
    res = run_bass_kernel_spmd(nc, in_maps, core_ids=list(range(N_CORES)))
    return _postprocess(inputs["x"], res.results, merge_info, meta)


# revision 13
# speedup vs baseline: 3.5602x; 1.1183x over previous
"""CGConv message-passing kernel for 8 Trainium2 NeuronCores.

Strategy (self-contained; shapes hardcoded for the nn_CGConv problem):
 - Sort edges by destination node (col); shard edges into 8 buckets by
   col-range so every core owns a disjoint slice of output nodes (no
   collective needed).
 - Host pre-gathers x[row]/x[col] as channel-major bf16 tables so the
   device streams them with plain contiguous DMA.
 - Per 512-edge tile the device computes gate/msg pre-activations with
   six N=512 matmuls (bias folded in via a ones-row on the edge_attr
   chunk), applies sigmoid x softplus via the natural_log_exp ACT table
   set only (exp(-g), exp(c), ln(t2+1); sigma = 1/(1+e^-g) on DVE), and
   segment-sums messages by destination via one-hot matmuls into a
   128-node window whose base is the tile's first (sorted) col.
 - Per-tile window sums go back to DRAM; the host merges windows,
   adds the residual, and assembles the full [25000, 128] output.
"""

import numpy as np
import ml_dtypes

BF16 = ml_dtypes.bfloat16

N_NODES = 25000
N_EDGES = 400000
C = 128
EC = 64
N_CORES = 8
SHARD = 3125          # nodes per core
SHARD_PAD = 3200      # padded (multiple of 128)
TILE = 512            # edges per super-tile


def _prep(x, edge_index, edge_attr, gate_w, gate_b, msg_w, msg_b):
    row = np.asarray(edge_index[0], dtype=np.int64)
    col = np.asarray(edge_index[1], dtype=np.int64)
    x = np.asarray(x, dtype=np.float32)
    edge_attr = np.asarray(edge_attr, dtype=np.float32)

    order = np.argsort(col, kind="stable")
    row_s, col_s = row[order], col[order]
    attr_s = edge_attr[order]

    shard_of = col_s // SHARD
    # bucket boundaries per core
    starts = np.searchsorted(shard_of, np.arange(N_CORES))
    ends = np.searchsorted(shard_of, np.arange(N_CORES), side="right")
    sizes = ends - starts
    e_pad = int(-(-sizes.max() // TILE) * TILE)
    n_sup = e_pad // TILE

    x_bf = x.astype(BF16)
    xT_bf = np.ascontiguousarray(x_bf.T)  # [128, N]

    # segment-window width: covers the max col-span of any 512-edge tile
    max_span = 0
    for i in range(N_CORES):
        c_l = col_s[starts[i]:ends[i]] - i * SHARD
        for t0 in range(0, len(c_l), TILE):
            seg = c_l[t0:t0 + TILE]
            max_span = max(max_span, int(seg[-1]) - int(seg[0]))
    Wseg = int(-(-(max_span + 3) // 32) * 32)  # pad-edge slack + round to 32
    Wseg = max(64, min(Wseg, 192))

    in_maps = []
    merge_info = []  # per core: (bases list, n_valid_edges)
    for i in range(N_CORES):
        sl = slice(starts[i], ends[i])
        r_i = row_s[sl]
        c_loc = (col_s[sl] - i * SHARD).astype(np.int64)
        a_i = attr_s[sl]
        n_i = sizes[i]
        pad = e_pad - n_i

        # pad col: must be >= SHARD (dropped at merge) and within W of its
        # tile's base. Real cols end <= 3124; pads go to a dedicated node id.
        if pad:
            last_real = int(c_loc[-1]) if n_i else 0
            pad_col = max(SHARD, last_real + 1)
            assert pad_col < SHARD_PAD
            r_i = np.concatenate([r_i, np.zeros(pad, np.int64)])
            c_loc = np.concatenate([c_loc, np.full(pad, pad_col, np.int64)])
            a_i = np.concatenate([a_i, np.zeros((pad, EC), np.float32)])

        # per-super-tile bases, clamped so base+W covers the tile's span
        bases = []
        for t in range(n_sup):
            seg = c_loc[t * TILE:(t + 1) * TILE]
            b = int(seg[0])
            assert int(seg[-1]) - b < Wseg, (
                f"core {i} tile {t}: col span {int(seg[-1]) - b} >= W={Wseg}")
            bases.append(b)
        bases_arr = np.repeat(np.asarray(bases, np.int64), TILE)

        xrowT = np.ascontiguousarray(x_bf[r_i].T)                  # [128, e_pad]
        # pad edges point past the real node range; clamp (their output is dropped)
        xcolT = np.ascontiguousarray(
            xT_bf[:, np.minimum(c_loc + i * SHARD, N_NODES - 1)])  # [128, e_pad]
        attrT = np.empty((EC + 1, e_pad), dtype=BF16)
        attrT[:EC] = a_i.T
        attrT[EC] = 1.0

        # one-hot segment matrices, host-built: B[e, w] = (col[e] - base == w)
        colsub = (c_loc - bases_arr).astype(np.int64)
        assert colsub.min() >= 0 and colsub.max() < Wseg
        bmat = np.zeros((e_pad, Wseg), dtype=BF16)
        bmat[np.arange(e_pad), colsub] = 1.0
        # [128 (edge-in-subtile), n_sub * W] so a tile's 4 B's are contiguous
        bmat = np.ascontiguousarray(
            bmat.reshape(e_pad // 128, 128, Wseg).transpose(1, 0, 2)
        ).reshape(128, -1)

        in_maps.append({
            "xrowT": xrowT,
            "xcolT": xcolT,
            "attrT": attrT,
            "bmat": bmat,
        })
        merge_info.append((bases, n_i))

    # shared (replicated) weight tables
    gw = np.asarray(gate_w, np.float32)
    mw = np.asarray(msg_w, np.float32)
    gb = np.asarray(gate_b, np.float32)
    mb = np.asarray(msg_b, np.float32)
    w1g = np.ascontiguousarray(gw[:, 0:128].T).astype(BF16)       # [128,128] lhsT
    w2g = np.ascontiguousarray(gw[:, 128:256].T).astype(BF16)
    no_bias = False
    w3g = np.empty((EC + 1, C), dtype=BF16)
    w3g[:EC] = gw[:, 256:320].T
    w3g[EC] = gb
    w3m = np.empty((EC + 1, C), dtype=BF16)
    w3m[:EC] = mw[:, 256:320].T
    w3m[EC] = mb
    w1m = np.ascontiguousarray(mw[:, 0:128].T).astype(BF16)
    w2m = np.ascontiguousarray(mw[:, 128:256].T).astype(BF16)

    ident = np.eye(128, dtype=BF16)
    shared = {"w1g": w1g, "w2g": w2g, "w3g": w3g,
              "w1m": w1m, "w2m": w2m, "w3m": w3m, "ident": ident}
    for m in in_maps:
        m.update(shared)

    meta = {"e_pad": e_pad, "n_sup": n_sup, "w_seg": Wseg, "no_bias": bool(no_bias)}
    return in_maps, meta, merge_info


def _build(meta):
    import concourse.bacc as bacc
    import concourse.mybir as mybir
    from concourse import tile

    e_pad, n_sup = meta["e_pad"], meta["n_sup"]
    W = meta["w_seg"]
    no_bias = meta["no_bias"]
    K3 = C if no_bias else EC + 1
    bf = mybir.dt.bfloat16
    f32 = mybir.dt.float32
    AF = mybir.ActivationFunctionType

    nc = bacc.Bacc(None, target_bir_lowering=False, debug=False)

    xrowT_d = nc.declare_dram_parameter("xrowT", [C, e_pad], bf, isOutput=False)
    xcolT_d = nc.declare_dram_parameter("xcolT", [C, e_pad], bf, isOutput=False)
    attrT_d = nc.declare_dram_parameter("attrT", [EC + 1, e_pad], bf, isOutput=False)
    bmat_d = nc.declare_dram_parameter("bmat", [128, (e_pad // 128) * W], bf, isOutput=False)
    ident_d = nc.declare_dram_parameter("ident", [128, 128], bf, isOutput=False)
    w_d = {n: nc.declare_dram_parameter(n, [128 if n[1] in "12" else K3, C], bf, isOutput=False)
           for n in ("w1g", "w2g", "w3g", "w1m", "w2m", "w3m")}
    wsums_d = nc.declare_dram_parameter("wsums", [C, n_sup * W], f32, isOutput=True)

    with tile.TileContext(nc) as tc:
        with (
            tc.tile_pool(name="const", bufs=1) as cpool,
            tc.tile_pool(name="stream", bufs=3) as spool,
            tc.tile_pool(name="elem", bufs=3) as epool,
            tc.tile_pool(name="bmat", bufs=3) as bpool,
            tc.tile_pool(name="gps", bufs=2, space="PSUM") as gate_pool,
            tc.tile_pool(name="mps", bufs=2, space="PSUM") as msg_pool,
            tc.tile_pool(name="tps", bufs=2, space="PSUM") as t_pool,
            tc.tile_pool(name="sps", bufs=2, space="PSUM") as s_pool,
        ):
            wt = {}
            ident = cpool.tile([128, 128], bf, tag="ident")
            nc.sync.dma_start(ident[:], ident_d[:])

            # ~8us of dense back-to-back matmuls: one continuous burst longer
            # than the HAM activity window so the PE clock unthrottles to
            # 2.4GHz before the real stream (which never idles long enough
            # to re-throttle, but also never bursts long enough to warm).
            warm_in = cpool.tile([128, TILE], bf, tag="warm")
            nc.gpsimd.memset(warm_in[:], 0.0)
            warm_ps = s_pool.tile([C, W], f32, tag="seg", name="warm_ps")
            for _ in range(90):
                nc.tensor.matmul(warm_ps[:], warm_in[:, 0:128], warm_in[:, 0:W],
                                 start=True, stop=True, skip_group_check=True)
            for n, d in w_d.items():
                if no_bias and n == "w3m":
                    continue
                kdim = 128 if n[1] in "12" else K3
                wt[n] = cpool.tile([kdim, C], bf, tag=n, name=n)
                nc.sync.dma_start(wt[n][:], d[:])

            for t in range(n_sup):
                esl = slice(t * TILE, (t + 1) * TILE)
                xrow_t = spool.tile([C, TILE], bf, tag="xrow")
                nc.sync.dma_start(xrow_t[:], xrowT_d[:, esl])
                xcol_t = spool.tile([C, TILE], bf, tag="xcol")
                nc.sync.dma_start(xcol_t[:], xcolT_d[:, esl])
                attr_t = spool.tile([EC + 1, TILE], bf, tag="attr")
                nc.sync.dma_start(attr_t[:], attrT_d[:, esl])

                gate_ps = gate_pool.tile([C, TILE], f32, tag="gate")
                msg_ps = msg_pool.tile([C, TILE], f32, tag="msg")
                nc.tensor.matmul(gate_ps[:], wt["w1g"][:], xrow_t[:], start=True, stop=False)
                nc.tensor.matmul(gate_ps[:], wt["w2g"][:], xcol_t[:], start=False, stop=False)
                nc.tensor.matmul(msg_ps[:], wt["w1m"][:], xrow_t[:], start=True, stop=False)
                nc.tensor.matmul(msg_ps[:], wt["w2m"][:], xcol_t[:], start=False, stop=False)
                nc.tensor.matmul(gate_ps[:], wt["w3g"][:], attr_t[:],
                                 start=False, stop=True)
                nc.tensor.matmul(msg_ps[:], wt["w3m"][:], attr_t[:],
                                 start=False, stop=True)

                # t1 = exp(-g), t2 = exp(c), sp = ln(t2 + 1)
                t1 = epool.tile([C, TILE], f32, tag="t1")
                nc.scalar.activation(t1[:], gate_ps[:], AF.Exp, scale=-1.0)
                t2 = epool.tile([C, TILE], bf, tag="t2")
                nc.scalar.activation(t2[:], msg_ps[:], AF.Exp)
                sp = epool.tile([C, TILE], f32, tag="sp")
                nc.scalar.activation(sp[:], t2[:], AF.Ln, bias=1.0)

                # m = sp / (1 + t1)
                wd = epool.tile([C, TILE], f32, tag="wd")
                nc.vector.tensor_scalar_add(wd[:], t1[:], 1.0)
                rc = epool.tile([C, TILE], f32, tag="rc")
                nc.vector.reciprocal_approx_fast(rc[:], wd[:])
                m = epool.tile([C, TILE], bf, tag="m")
                nc.vector.tensor_mul(m[:], sp[:], rc[:])

                # transpose m -> [edges, ch] on the PE
                mt_ps = t_pool.tile([128, TILE], bf, tag="mt")
                for j in range(4):
                    nc.tensor.transpose(mt_ps[:, j * 128:(j + 1) * 128],
                                        m[:, j * 128:(j + 1) * 128], ident[:])
                m_t = epool.tile([128, TILE], bf, tag="m_t")
                nc.vector.tensor_copy(m_t[:], mt_ps[:])

                # segment windows: host-built one-hot B, shared base per tile
                seg_ps = s_pool.tile([C, W], f32, tag="seg")
                b_t = bpool.tile([128, 4 * W], bf, tag="b")
                nc.sync.dma_start(b_t[:], bmat_d[:, t * 4 * W:(t + 1) * 4 * W])
                for j in range(4):
                    nc.tensor.matmul(seg_ps[:], m_t[:, j * 128:(j + 1) * 128],
                                     b_t[:, j * W:(j + 1) * W],
                                     start=(j == 0), stop=(j == 3))
                wsum = epool.tile([C, W], f32, tag="wsum")
                nc.vector.tensor_copy(wsum[:], seg_ps[:])
                nc.sync.dma_start(wsums_d[:, t * W:(t + 1) * W], wsum[:])

    # Force every activation onto natural_log_exp_and_others (exp AND ln):
    # the stock chooser greedily alternates exp_and_others/natural_log,
    # inserting ~2 table loads (~2.6us) per tile.
    import concourse.bacc as _bacc
    real_get = _bacc.get_activation_tables

    def pinned_tables(arch):
        tabs = real_get(arch)
        return {name: (funcs if name == "natural_log_exp_and_others" else set())
                for name, funcs in tabs.items()}

    _bacc.get_activation_tables = pinned_tables
    try:
        nc.compile()
    finally:
        _bacc.get_activation_tables = real_get
    return nc


def _postprocess(x, results, merge_info, meta):
    n_sup = meta["n_sup"]
    W = meta["w_seg"]
    out = np.empty((N_NODES, C), dtype=np.float32)
    for i in range(N_CORES):
        wsums = np.asarray(results[i]["wsums"], np.float32).reshape(C, n_sup, W)
        agg = np.zeros((C, SHARD_PAD + W), dtype=np.float32)
        bases, _ = merge_info[i]
        for t in range(n_sup):
            b = bases[t]
            agg[:, b:b + W] += wsums[:, t, :]
        out[i * SHARD:(i + 1) * SHARD] = agg[:, :SHARD].T
    out += np.asarray(x, np.float32)
    return out


_CACHE = {}


def kernel(**inputs):
    from concourse.bass_utils import run_bass_kernel_spmd

    in_maps, meta, merge_info = _prep(**inputs)
    key = (meta["e_pad"],)
    if key not in _CACHE:
        _CACHE[key] = _build(meta)
    nc = _CACHE[key]
    res = run_bass_kernel_spmd(nc, in_maps, core_ids=list(range(N_CORES)))
    return _postprocess(inputs["x"], res.results, merge_info, meta)
